# revision 4
# baseline (speedup 1.0000x reference)
"""GCN layer kernel for Trainium2 (8 NeuronCores, SPMD).

Computes relu(A_hat @ (H @ W) + b) as relu((A_hat @ H) @ W + b)
(segment-sum commutes with the dense feature transform).

Strategy:
  - Destination nodes sharded across 8 cores (12500 rows each); edges
    pre-partitioned by destination on the host.
  - Per core, edges are organized into 4 source-bucket streams (25000
    source rows per bucket -> int16 gather indices), ordered by
    destination window (128 rows). Per-(window, bucket) segment lengths
    are the max over cores so one SPMD program serves all cores; chunk
    boundaries (128 edges) may straddle windows - straddling chunks are
    processed once per window with host-masked (r, val) streams.
  - Gather: dma_gather in 2048-index calls on 4 SWDGE queues
    (HBM random-row reads are latency-bound; ~95 GB/s/core aggregate).
  - Per gather call the fp32 rows are cast to bf16 with one big DVE copy.
  - Per processing (window w, chunk): one fused DVE tensor_scalar builds
    S^T[e, d] = val_e * (r_e == d) in bf16; the tensor engine accumulates
    agg^T[f, d] += G^T S^T in PSUM (K = 128 edges, single-pass bf16).
  - Per window: agg^T -> SBUF (bf16), final matmul with W, bias + ReLU,
    DMA out.
"""

import os

import numpy as np

N = 100000
E = 1600000
F = 64
NCORES = 8
ND = N // NCORES          # 12500 destination rows per core
WSZ = 128                 # window = 128 destination rows
NW = (ND + WSZ - 1) // WSZ  # 98 windows (last has 84 rows)
NBUCK = 4                 # source buckets (int16 index limit)
BSZ = N // NBUCK          # 25000 source rows per bucket
CALL = 2048               # gather indices per dma_gather call
CCOL = CALL // 128        # 16 chunk-columns per call
NQ = 4                    # SWDGE gather queues

_cache = {}


def _preprocess(edge_row, edge_col, edge_vals):
    core = edge_row // ND
    r_local = edge_row - core * ND
    w = r_local // WSZ
    r_in_w = (r_local - w * WSZ).astype(np.float32)
    b = edge_col // BSZ
    col_local = (edge_col - b * BSZ).astype(np.int16)

    key = (core.astype(np.int64) * NW + w) * NBUCK + b
    counts = np.bincount(key, minlength=NCORES * NW * NBUCK).reshape(
        NCORES, NW, NBUCK)
    L = counts.max(axis=0)            # [NW, NBUCK] common segment lengths
    L[:, 0] = np.maximum(L[:, 0], 1)

    # common positions of (w, b) segments inside each bucket stream
    pos = np.zeros((NW, NBUCK), dtype=np.int64)
    pos[1:] = np.cumsum(L[:-1], axis=0)
    slen = pos[-1] + L[-1]                      # stream length per bucket
    nchunks = (slen + 127) // 128               # chunks per bucket
    plen = nchunks * 128                        # padded stream length
    cbase = np.zeros(NBUCK + 1, dtype=np.int64)  # chunk base per bucket
    cbase[1:] = np.cumsum(nchunks)
    tgc = int(cbase[-1])                        # total gather chunks

    # edge placement: position within its (c, w, b) segment
    order = np.argsort(key, kind="stable")
    seg_sizes = counts.reshape(-1)
    seg_off = np.zeros_like(seg_sizes)
    np.cumsum(seg_sizes[:-1], out=seg_off[1:])
    rank_sorted = np.arange(E, dtype=np.int64) - np.repeat(seg_off, seg_sizes)
    rank = np.empty(E, dtype=np.int64)
    rank[order] = rank_sorted
    p_edge = pos[w, b] + rank                   # position in bucket stream

    # per-core per-bucket position arrays
    idx_all = np.zeros((NCORES, NBUCK, int(plen.max())), dtype=np.int16)
    rpos = np.zeros((NCORES, NBUCK, int(plen.max())), dtype=np.float32)
    vpos = np.zeros((NCORES, NBUCK, int(plen.max())), dtype=np.float32)
    wpos = np.full((NBUCK, int(plen.max())), -1, dtype=np.int64)
    for bb in range(NBUCK):
        seg_id = np.repeat(np.arange(NW), L[:, bb])
        wpos[bb, :slen[bb]] = seg_id
    idx_all[core, b, p_edge] = col_local
    rpos[core, b, p_edge] = r_in_w
    vpos[core, b, p_edge] = edge_vals

    # processing list: (w, b, chunk j within bucket)
    procs = []          # (w, b, global_chunk)
    win_procs = []      # per window: (start, count) into procs
    for ww in range(NW):
        s = len(procs)
        for bb in range(NBUCK):
            if L[ww, bb] == 0:
                continue
            j0 = int(pos[ww, bb]) // 128
            j1 = int(pos[ww, bb] + L[ww, bb] - 1) // 128
            for j in range(j0, j1 + 1):
                procs.append((ww, bb, int(cbase[bb]) + j))
        win_procs.append((s, len(procs) - s))
    tp = len(procs)

    # host-masked one-hot parameter streams, [128, tp]
    pw = np.array([p[0] for p in procs], dtype=np.int64)
    pb = np.array([p[1] for p in procs], dtype=np.int64)
    pj = np.array([p[2] for p in procs], dtype=np.int64)  # global chunk
    local_j = pj - cbase[pb]
    slot_pos = local_j[:, None] * 128 + np.arange(128)[None, :]  # [tp,128]
    mask = wpos[pb[:, None], slot_pos] == pw[:, None]
    rr_all = np.zeros((NCORES, 128, tp), dtype=np.float32)
    vv_all = np.zeros((NCORES, 128, tp), dtype=np.float32)
    for c in range(NCORES):
        rsel = rpos[c][pb[:, None], slot_pos]      # [tp, 128]
        vsel = vpos[c][pb[:, None], slot_pos]
        rr_all[c] = np.where(mask, rsel, 0.0).T
        vv_all[c] = np.where(mask, vsel, 0.0).T

    # gather calls: per bucket, CALL indices each (last smaller)
    calls = []   # (bucket, global col start, ncols)
    for bb in range(NBUCK):
        j = 0
        while j < nchunks[bb]:
            cc = min(CCOL, int(nchunks[bb]) - j)
            calls.append((bb, int(cbase[bb]) + j, cc))
            j += cc
    # interleave calls round-robin across buckets for queue balance
    by_b = [[cl for cl in calls if cl[0] == bb] for bb in range(NBUCK)]
    calls = []
    k = 0
    while any(by_b):
        for bb in range(NBUCK):
            if by_b[bb]:
                calls.append(by_b[bb].pop(0))
        k += 1

    # wrapped int16 index stream, [128, tgc * 8] (call-local wrapping)
    gidx = np.zeros((NCORES, 128, tgc * 8), dtype=np.int16)
    for bb, gc0, cc in calls:
        bb_j0 = (gc0 - int(cbase[bb])) * 128
        ncols8 = cc * 8
        sl = idx_all[:, bb, bb_j0:bb_j0 + cc * 128].reshape(
            NCORES, cc * 128 // 16, 16)
        wrapped = np.swapaxes(sl, 1, 2)
        gidx[:, :, gc0 * 8:gc0 * 8 + ncols8] = np.tile(wrapped, (1, 8, 1))

    layout = (
        tuple(int(x) for x in nchunks),
        tuple(calls),
        tuple((int(s), int(n)) for s, n in win_procs),
        tuple((int(a), int(bb), int(c)) for a, bb, c in procs),
        tp, tgc,
    )
    return layout, gidx, rr_all, vv_all


def _install_trace_hook():
    import sys
    import types

    if "antenv.axon_hooks" in sys.modules:
        return
    mod = types.ModuleType("antenv.axon_hooks")

    def set_hook(h):
        mod._hook = h

    def get_hook():
        return getattr(mod, "_hook", None)

    mod.set_axon_ntff_profile_hook = set_hook
    mod.get_axon_ntff_profile_hook = get_hook
    sys.modules["antenv.axon_hooks"] = mod
    try:
        from trn_agent_boot.trn_boot import _ntff_profile_via_ctypes

        mod._hook = _ntff_profile_via_ctypes("/opt/axon/libaxon_pjrt.so")
    except Exception:
        mod._hook = None


def _build(layout):
    import concourse.mybir as mybir
    import concourse.tile as tile
    from concourse import bacc

    nchunks, calls, win_procs, procs, tp, tgc = layout

    nc = bacc.Bacc("TRN2", target_bir_lowering=False, debug=False,
                   num_devices=NCORES, num_swdge_queues=NQ)
    f32 = mybir.dt.float32
    bf16 = mybir.dt.bfloat16
    src = nc.dram_tensor("src", [N, F], f32, kind="ExternalInput").ap()
    gidx = nc.dram_tensor("gidx", [128, tgc * 8], mybir.dt.int16,
                          kind="ExternalInput").ap()
    rr = nc.dram_tensor("rr", [128, tp], f32, kind="ExternalInput").ap()
    vv = nc.dram_tensor("vv", [128, tp], f32, kind="ExternalInput").ap()
    iota = nc.dram_tensor("iota", [128, 128], bf16, kind="ExternalInput").ap()
    wmat = nc.dram_tensor("wmat", [F, F], bf16, kind="ExternalInput").ap()
    bias = nc.dram_tensor("bias", [128, F], f32, kind="ExternalInput").ap()
    out = nc.dram_tensor("out", [ND, F], f32, kind="ExternalOutput").ap()

    # map global chunk -> (call index, col within call)
    chunk_call = {}
    for cix, (bb, gc0, cc) in enumerate(calls):
        for j in range(cc):
            chunk_call[gc0 + j] = (cix, j)

    with tile.TileContext(nc) as tc:
        with (
            tc.tile_pool(name="const", bufs=1) as constp,
            tc.tile_pool(name="meta", bufs=3) as metap,
            tc.tile_pool(name="gat", bufs=6) as gatp,
            tc.tile_pool(name="gbf", bufs=24) as gbfp,
            tc.tile_pool(name="st", bufs=6) as stp,
            tc.tile_pool(name="ps1", bufs=3, space="PSUM") as ps1p,
            tc.tile_pool(name="ps2", bufs=2, space="PSUM") as ps2p,
            tc.tile_pool(name="agg", bufs=3) as aggp,
            tc.tile_pool(name="ob", bufs=4) as obp,
        ):
            iota_t = constp.tile([128, 128], bf16)
            nc.sync.dma_start(out=iota_t[:], in_=iota[:])
            w_t = constp.tile([F, F], bf16)
            nc.sync.dma_start(out=w_t[:], in_=wmat[:])
            bias_t = constp.tile([128, F], f32)
            nc.sync.dma_start(out=bias_t[:], in_=bias[:])

            # metadata loads in slices of ~16 windows' processings
            MS = 512
            rr_tiles = {}
            for s0 in range(0, tp, MS):
                sl = min(MS, tp - s0)
                rr_t = metap.tile([128, MS], f32, tag="rr")
                nc.sync.dma_start(out=rr_t[:, :sl], in_=rr[:, s0:s0 + sl])
                vv_t = metap.tile([128, MS], f32, tag="vv")
                nc.sync.dma_start(out=vv_t[:, :sl], in_=vv[:, s0:s0 + sl])
                rr_tiles[s0] = (rr_t, vv_t)

            # gather + cast, call granularity
            gbf_tiles = []
            for cix, (bb, gc0, cc) in enumerate(calls):
                idx_t = metap.tile([128, CCOL * 8], mybir.dt.int16, tag="idx")
                nc.sync.dma_start(out=idx_t[:, :cc * 8],
                                  in_=gidx[:, gc0 * 8:(gc0 + cc) * 8])
                g_t = gatp.tile([128, CCOL, F], f32, tag="g")
                nidx = cc * 128
                nc.gpsimd.dma_gather(
                    g_t[:, :cc, :],
                    src[bb * BSZ:(bb + 1) * BSZ, :],
                    idx_t[:, :nidx // 16],
                    num_idxs=nidx, num_idxs_reg=nidx, elem_size=F,
                    single_packet=False, queue_num=bb % NQ,
                )
                gb_t = gbfp.tile([128, CCOL, F], bf16, tag="gb")
                nc.vector.tensor_copy(out=gb_t[:, :cc, :], in_=g_t[:, :cc, :])
                gbf_tiles.append(gb_t)

            for ww, (ps, pn) in enumerate(win_procs):
                ps1 = ps1p.tile([F, 128], f32, space="PSUM")
                for k in range(pn):
                    p = ps + k
                    _, _, gc = procs[p]
                    cix, colj = chunk_call[gc]
                    s0 = (p // MS) * MS
                    rr_t, vv_t = rr_tiles[s0]
                    st = stp.tile([128, 128], bf16)
                    nc.vector.tensor_scalar(
                        out=st[:], in0=iota_t[:],
                        scalar1=rr_t[:, p - s0:p - s0 + 1],
                        scalar2=vv_t[:, p - s0:p - s0 + 1],
                        op0=mybir.AluOpType.is_equal,
                        op1=mybir.AluOpType.mult,
                    )
                    nc.tensor.matmul(
                        out=ps1[:], lhsT=gbf_tiles[cix][:, colj, :], rhs=st[:],
                        start=(k == 0), stop=(k == pn - 1),
                    )
                aggT = aggp.tile([F, 128], bf16)
                nc.vector.tensor_copy(out=aggT[:], in_=ps1[:])
                ps2 = ps2p.tile([128, F], f32, space="PSUM")
                nc.tensor.matmul(out=ps2[:], lhsT=aggT[:], rhs=w_t[:],
                                 start=True, stop=True)
                ob = obp.tile([128, F], f32, tag="ob")
                nc.vector.tensor_tensor(out=ob[:], in0=ps2[:], in1=bias_t[:],
                                        op=mybir.AluOpType.add)
                ob2 = obp.tile([128, F], f32, tag="ob2")
                nc.scalar.activation(ob2[:], ob[:],
                                     mybir.ActivationFunctionType.Relu)
                rows = min(WSZ, ND - ww * WSZ)
                nc.scalar.dma_start(out=out[ww * WSZ:ww * WSZ + rows, :],
                                    in_=ob2[:rows, :])
    nc.compile()
    return nc


def kernel(node_features, edge_row, edge_col, edge_vals, kernel, bias):
    from concourse.bass_utils import run_bass_kernel_spmd
    import ml_dtypes

    trace = os.environ.get("GCN_TRACE", "") == "1"
    if trace:
        _install_trace_hook()

    node_features = np.ascontiguousarray(node_features, dtype=np.float32)
    edge_row = np.asarray(edge_row, dtype=np.int64)
    edge_col = np.asarray(edge_col, dtype=np.int64)
    edge_vals = np.ascontiguousarray(edge_vals, dtype=np.float32)
    wmat = np.ascontiguousarray(kernel, dtype=ml_dtypes.bfloat16)
    bias_v = np.asarray(bias, dtype=np.float32)

    layout, gidx, rr_all, vv_all = _preprocess(edge_row, edge_col, edge_vals)

    key = hash(repr(layout))
    if key not in _cache:
        _cache[key] = _build(layout)
    nc = _cache[key]

    iota_v = np.ascontiguousarray(np.tile(
        np.arange(128, dtype=np.float32)[None, :], (128, 1))).astype(
        ml_dtypes.bfloat16)
    bias_b = np.ascontiguousarray(np.tile(bias_v[None, :], (128, 1)))
    in_maps = []
    for c in range(NCORES):
        in_maps.append({
            "src": node_features,
            "gidx": np.ascontiguousarray(gidx[c]),
            "rr": np.ascontiguousarray(rr_all[c]),
            "vv": np.ascontiguousarray(vv_all[c]),
            "iota": iota_v,
            "wmat": wmat,
            "bias": bias_b,
        })
    res = run_bass_kernel_spmd(nc, in_maps, core_ids=list(range(NCORES)),
                               trace=trace)
    if trace and res.exec_time_ns is not None:
        print(f"HW exec time: {res.exec_time_ns} ns")
        globals()["_last_exec_ns"] = res.exec_time_ns
        globals()["_last_trace"] = (res.instructions_and_trace or (None, None))[1]
    return np.concatenate([res.results[c]["out"] for c in range(NCORES)],
                          axis=0)


# revision 6
# speedup vs baseline: 1.1323x; 1.1323x over previous
"""GCN layer kernel for Trainium2 (8 NeuronCores, SPMD).

Computes relu(A_hat @ (H @ W) + b) as relu((A_hat @ H) @ W + b)
(segment-sum commutes with the dense feature transform).

Strategy:
  - Destination nodes sharded across 8 cores (12500 rows each); edges
    pre-partitioned by destination on the host.
  - Per core, edges are organized into 4 source-bucket streams (25000
    source rows per bucket -> int16 gather indices), ordered by
    destination window (128 rows). Per-(window, bucket) segment lengths
    are the max over cores so one SPMD program serves all cores; chunk
    boundaries (128 edges) may straddle windows - straddling chunks are
    processed once per window with host-masked (r, val) streams.
  - Gather: dma_gather in 2048-index calls on 4 SWDGE queues
    (HBM random-row reads are latency-bound; ~95 GB/s/core aggregate).
  - Per gather call the fp32 rows are cast to bf16 with one big DVE copy.
  - Per processing (window w, chunk): one fused DVE tensor_scalar builds
    S^T[e, d] = val_e * (r_e == d) in bf16; the tensor engine accumulates
    agg^T[f, d] += G^T S^T in PSUM (K = 128 edges, single-pass bf16).
  - Per window: agg^T -> SBUF (bf16), final matmul with W, bias + ReLU,
    DMA out.
"""

import os

import numpy as np

N = 100000
E = 1600000
F = 64
NCORES = 8
ND = N // NCORES          # 12500 destination rows per core
WSZ = 128                 # window = 128 destination rows
NW = (ND + WSZ - 1) // WSZ  # 98 windows (last has 84 rows)
NBUCK = 4                 # source buckets (int16 index limit)
BSZ = N // NBUCK          # 25000 source rows per bucket
CALL = 2048               # gather indices per dma_gather call
CCOL = CALL // 128        # 16 chunk-columns per call
NQ = 4                    # SWDGE gather queues

_cache = {}


def _preprocess(edge_row, edge_col, edge_vals):
    core = edge_row // ND
    r_local = edge_row - core * ND
    w = r_local // WSZ
    r_in_w = (r_local - w * WSZ).astype(np.float32)
    b = edge_col // BSZ
    col_local = (edge_col - b * BSZ).astype(np.int16)

    key = (core.astype(np.int64) * NW + w) * NBUCK + b
    counts = np.bincount(key, minlength=NCORES * NW * NBUCK).reshape(
        NCORES, NW, NBUCK)
    L = counts.max(axis=0)            # [NW, NBUCK] common segment lengths
    L[:, 0] = np.maximum(L[:, 0], 1)

    # common positions of (w, b) segments inside each bucket stream
    pos = np.zeros((NW, NBUCK), dtype=np.int64)
    pos[1:] = np.cumsum(L[:-1], axis=0)
    slen = pos[-1] + L[-1]                      # stream length per bucket
    nchunks = (slen + 127) // 128               # chunks per bucket
    plen = nchunks * 128                        # padded stream length
    cbase = np.zeros(NBUCK + 1, dtype=np.int64)  # chunk base per bucket
    cbase[1:] = np.cumsum(nchunks)
    tgc = int(cbase[-1])                        # total gather chunks

    # edge placement: position within its (c, w, b) segment
    order = np.argsort(key, kind="stable")
    seg_sizes = counts.reshape(-1)
    seg_off = np.zeros_like(seg_sizes)
    np.cumsum(seg_sizes[:-1], out=seg_off[1:])
    rank_sorted = np.arange(E, dtype=np.int64) - np.repeat(seg_off, seg_sizes)
    rank = np.empty(E, dtype=np.int64)
    rank[order] = rank_sorted
    p_edge = pos[w, b] + rank                   # position in bucket stream

    # per-core per-bucket position arrays
    idx_all = np.zeros((NCORES, NBUCK, int(plen.max())), dtype=np.int16)
    rpos = np.zeros((NCORES, NBUCK, int(plen.max())), dtype=np.float32)
    vpos = np.zeros((NCORES, NBUCK, int(plen.max())), dtype=np.float32)
    wpos = np.full((NBUCK, int(plen.max())), -1, dtype=np.int64)
    for bb in range(NBUCK):
        seg_id = np.repeat(np.arange(NW), L[:, bb])
        wpos[bb, :slen[bb]] = seg_id
    idx_all[core, b, p_edge] = col_local
    rpos[core, b, p_edge] = r_in_w
    vpos[core, b, p_edge] = edge_vals

    # processing list: (w, b, chunk j within bucket)
    procs = []          # (w, b, global_chunk)
    win_procs = []      # per window: (start, count) into procs
    for ww in range(NW):
        s = len(procs)
        for bb in range(NBUCK):
            if L[ww, bb] == 0:
                continue
            j0 = int(pos[ww, bb]) // 128
            j1 = int(pos[ww, bb] + L[ww, bb] - 1) // 128
            for j in range(j0, j1 + 1):
                procs.append((ww, bb, int(cbase[bb]) + j))
        win_procs.append((s, len(procs) - s))
    tp = len(procs)

    # host-masked one-hot parameter streams, [128, tp]
    pw = np.array([p[0] for p in procs], dtype=np.int64)
    pb = np.array([p[1] for p in procs], dtype=np.int64)
    pj = np.array([p[2] for p in procs], dtype=np.int64)  # global chunk
    local_j = pj - cbase[pb]
    slot_pos = local_j[:, None] * 128 + np.arange(128)[None, :]  # [tp,128]
    mask = wpos[pb[:, None], slot_pos] == pw[:, None]
    rr_all = np.zeros((NCORES, 128, tp), dtype=np.float32)
    vv_all = np.zeros((NCORES, 128, tp), dtype=np.float32)
    for c in range(NCORES):
        rsel = rpos[c][pb[:, None], slot_pos]      # [tp, 128]
        vsel = vpos[c][pb[:, None], slot_pos]
        rr_all[c] = np.where(mask, rsel, 0.0).T
        vv_all[c] = np.where(mask, vsel, 0.0).T

    # gather calls: per bucket, CALL indices each (last smaller)
    calls = []   # (bucket, global col start, ncols)
    for bb in range(NBUCK):
        j = 0
        while j < nchunks[bb]:
            cc = min(CCOL, int(nchunks[bb]) - j)
            calls.append((bb, int(cbase[bb]) + j, cc))
            j += cc
    # interleave calls round-robin across buckets for queue balance
    by_b = [[cl for cl in calls if cl[0] == bb] for bb in range(NBUCK)]
    calls = []
    k = 0
    while any(by_b):
        for bb in range(NBUCK):
            if by_b[bb]:
                calls.append(by_b[bb].pop(0))
        k += 1

    # wrapped int16 index stream, [128, tgc * 8] (call-local wrapping)
    gidx = np.zeros((NCORES, 128, tgc * 8), dtype=np.int16)
    for bb, gc0, cc in calls:
        bb_j0 = (gc0 - int(cbase[bb])) * 128
        ncols8 = cc * 8
        sl = idx_all[:, bb, bb_j0:bb_j0 + cc * 128].reshape(
            NCORES, cc * 128 // 16, 16)
        wrapped = np.swapaxes(sl, 1, 2)
        gidx[:, :, gc0 * 8:gc0 * 8 + ncols8] = np.tile(wrapped, (1, 8, 1))

    layout = (
        tuple(int(x) for x in nchunks),
        tuple(calls),
        tuple((int(s), int(n)) for s, n in win_procs),
        tuple((int(a), int(bb), int(c)) for a, bb, c in procs),
        tp, tgc,
    )
    return layout, gidx, rr_all, vv_all


def _install_trace_hook():
    import sys
    import types

    if "antenv.axon_hooks" in sys.modules:
        return
    mod = types.ModuleType("antenv.axon_hooks")

    def set_hook(h):
        mod._hook = h

    def get_hook():
        return getattr(mod, "_hook", None)

    mod.set_axon_ntff_profile_hook = set_hook
    mod.get_axon_ntff_profile_hook = get_hook
    sys.modules["antenv.axon_hooks"] = mod
    try:
        from trn_agent_boot.trn_boot import _ntff_profile_via_ctypes

        mod._hook = _ntff_profile_via_ctypes("/opt/axon/libaxon_pjrt.so")
    except Exception:
        mod._hook = None


def _build(layout):
    import concourse.mybir as mybir
    import concourse.tile as tile
    from concourse import bacc

    nchunks, calls, win_procs, procs, tp, tgc = layout

    nc = bacc.Bacc("TRN2", target_bir_lowering=False, debug=False,
                   num_devices=NCORES, num_swdge_queues=NQ)
    f32 = mybir.dt.float32
    bf16 = mybir.dt.bfloat16
    src = nc.dram_tensor("src", [N, F], f32, kind="ExternalInput").ap()
    gidx = nc.dram_tensor("gidx", [128, tgc * 8], mybir.dt.int16,
                          kind="ExternalInput").ap()
    rr = nc.dram_tensor("rr", [128, tp], f32, kind="ExternalInput").ap()
    vv = nc.dram_tensor("vv", [128, tp], f32, kind="ExternalInput").ap()
    iota = nc.dram_tensor("iota", [128, 128], bf16, kind="ExternalInput").ap()
    wmat = nc.dram_tensor("wmat", [F + 1, F], bf16, kind="ExternalInput").ap()
    out = nc.dram_tensor("out", [ND, F], f32, kind="ExternalOutput").ap()

    # map global chunk -> (call index, col within call)
    chunk_call = {}
    for cix, (bb, gc0, cc) in enumerate(calls):
        for j in range(cc):
            chunk_call[gc0 + j] = (cix, j)

    with tile.TileContext(nc) as tc:
        with (
            tc.tile_pool(name="const", bufs=1) as constp,
            tc.tile_pool(name="meta", bufs=3) as metap,
            tc.tile_pool(name="gat", bufs=10) as gatp,
            tc.tile_pool(name="gbf", bufs=24) as gbfp,
            tc.tile_pool(name="st", bufs=12) as stp,
            tc.tile_pool(name="ps1", bufs=3, space="PSUM") as ps1p,
            tc.tile_pool(name="ps2", bufs=2, space="PSUM") as ps2p,
            tc.tile_pool(name="agg", bufs=3) as aggp,
            tc.tile_pool(name="ob", bufs=4) as obp,
        ):
            iota_t = constp.tile([128, 128], bf16)
            nc.sync.dma_start(out=iota_t[:], in_=iota[:])
            w_t = constp.tile([F + 1, F], bf16)
            nc.sync.dma_start(out=w_t[:], in_=wmat[:])

            # metadata loads in slices of ~16 windows' processings
            MS = 512
            rr_tiles = {}
            for s0 in range(0, tp, MS):
                sl = min(MS, tp - s0)
                rr_t = metap.tile([128, MS], f32, tag="rr")
                nc.sync.dma_start(out=rr_t[:, :sl], in_=rr[:, s0:s0 + sl])
                vv_t = metap.tile([128, MS], f32, tag="vv")
                nc.sync.dma_start(out=vv_t[:, :sl], in_=vv[:, s0:s0 + sl])
                rr_tiles[s0] = (rr_t, vv_t)

            # gather + cast, call granularity
            gbf_tiles = []
            for cix, (bb, gc0, cc) in enumerate(calls):
                idx_t = metap.tile([128, CCOL * 8], mybir.dt.int16, tag="idx")
                nc.sync.dma_start(out=idx_t[:, :cc * 8],
                                  in_=gidx[:, gc0 * 8:(gc0 + cc) * 8])
                g_t = gatp.tile([128, CCOL, F], f32, tag="g")
                nidx = cc * 128
                nc.gpsimd.dma_gather(
                    g_t[:, :cc, :],
                    src[bb * BSZ:(bb + 1) * BSZ, :],
                    idx_t[:, :nidx // 16],
                    num_idxs=nidx, num_idxs_reg=nidx, elem_size=F,
                    single_packet=False, queue_num=bb % NQ,
                )
                gb_t = gbfp.tile([128, CCOL, F], bf16, tag="gb")
                nc.scalar.copy(out=gb_t[:, :cc, :], in_=g_t[:, :cc, :])
                gbf_tiles.append(gb_t)

            for ww, (ps, pn) in enumerate(win_procs):
                ps1 = ps1p.tile([F, 128], f32, space="PSUM")
                for k in range(pn):
                    p = ps + k
                    _, _, gc = procs[p]
                    cix, colj = chunk_call[gc]
                    s0 = (p // MS) * MS
                    rr_t, vv_t = rr_tiles[s0]
                    st = stp.tile([128, 128], bf16)
                    nc.vector.tensor_scalar(
                        out=st[:], in0=iota_t[:],
                        scalar1=rr_t[:, p - s0:p - s0 + 1],
                        scalar2=vv_t[:, p - s0:p - s0 + 1],
                        op0=mybir.AluOpType.is_equal,
                        op1=mybir.AluOpType.mult,
                    )
                    nc.tensor.matmul(
                        out=ps1[:], lhsT=gbf_tiles[cix][:, colj, :], rhs=st[:],
                        start=(k == 0), stop=(k == pn - 1),
                    )
                aggT = aggp.tile([F + 1, 128], bf16)
                nc.vector.tensor_copy(out=aggT[:F, :], in_=ps1[:])
                nc.vector.memset(aggT[F:F + 1, :], 1.0)
                ps2 = ps2p.tile([128, F], f32, space="PSUM")
                nc.tensor.matmul(out=ps2[:], lhsT=aggT[:], rhs=w_t[:],
                                 start=True, stop=True)
                ob2 = obp.tile([128, F], f32, tag="ob2")
                nc.scalar.activation(ob2[:], ps2[:],
                                     mybir.ActivationFunctionType.Relu)
                rows = min(WSZ, ND - ww * WSZ)
                nc.scalar.dma_start(out=out[ww * WSZ:ww * WSZ + rows, :],
                                    in_=ob2[:rows, :])
    nc.compile()
    return nc


def kernel(node_features, edge_row, edge_col, edge_vals, kernel, bias):
    from concourse.bass_utils import run_bass_kernel_spmd
    import ml_dtypes

    trace = os.environ.get("GCN_TRACE", "") == "1"
    if trace:
        _install_trace_hook()

    node_features = np.ascontiguousarray(node_features, dtype=np.float32)
    edge_row = np.asarray(edge_row, dtype=np.int64)
    edge_col = np.asarray(edge_col, dtype=np.int64)
    edge_vals = np.ascontiguousarray(edge_vals, dtype=np.float32)
    w65 = np.vstack([np.asarray(kernel, dtype=np.float32),
                     np.asarray(bias, dtype=np.float32)[None, :]])
    wmat = np.ascontiguousarray(w65, dtype=ml_dtypes.bfloat16)

    layout, gidx, rr_all, vv_all = _preprocess(edge_row, edge_col, edge_vals)

    key = hash(repr(layout))
    if key not in _cache:
        _cache[key] = _build(layout)
    nc = _cache[key]

    iota_v = np.ascontiguousarray(np.tile(
        np.arange(128, dtype=np.float32)[None, :], (128, 1))).astype(
        ml_dtypes.bfloat16)
    in_maps = []
    for c in range(NCORES):
        in_maps.append({
            "src": node_features,
            "gidx": np.ascontiguousarray(gidx[c]),
            "rr": np.ascontiguousarray(rr_all[c]),
            "vv": np.ascontiguousarray(vv_all[c]),
            "iota": iota_v,
            "wmat": wmat,
        })
    res = run_bass_kernel_spmd(nc, in_maps, core_ids=list(range(NCORES)),
                               trace=trace)
    if trace and res.exec_time_ns is not None:
        print(f"HW exec time: {res.exec_time_ns} ns")
        globals()["_last_exec_ns"] = res.exec_time_ns
        globals()["_last_trace"] = (res.instructions_and_trace or (None, None))[1]
    return np.concatenate([res.results[c]["out"] for c in range(NCORES)],
                          axis=0)
